# revision 8
# baseline (speedup 1.0000x reference)
"""Trainium2 Bass kernel for the SE-gated Non-local block (rank-1 attention).

Math (per batch item b, x viewed as [C, N] with N = H*W):
    S[c]    = sum_n x[c, n]                      (spatial sum)
    hid     = relu((se_w1 / N) @ S + se_b1)      (SE bottleneck; 1/N folds the mean)
    gate    = sigmoid(w2e @ [hid; 1])            (se_b2 folded in as an extra w2 row)
    w5e     = gate * [theta_w | 0 | 0 | g_w | phi_w]  [C, 5]
    prow    = w5e.T @ x + [th_b, 1, 1, g_b, phi_b]    [5, N]
              rows: theta, ONES, ONES, g, phi   (the ones rows come from the bias)
    s_raw   = sum_n prow[3] * prow[4]
    out     = x + As (outer) theta + (Bc_hi + Bc_lo) (outer) ones   where
              inv = bn_gamma / sqrt(bn_var + eps)
              As  = (W_w * inv / N) * s_raw      (1/N folds the f/N normalizer)
              Bc  = (W_b - bn_mean) * inv + bn_beta,  split hi/lo bf16.

Schedule (v2): HBM traffic is the floor (read x + write out = 37.7 MB/core at
~410 GB/s observed = ~92 us of ring time).  Everything is arranged so the ONE
sync HWDGE ring is busy end-to-end: 8 sequential chunk loads [~8..54us], then
8 chunk stores queued right behind [~56..102us].

- x is resident in SBUF as bf16 (9 KB/partition/chunk); the f32 loads land in
  a small transient pool (3 bufs) and are released by the fused ACT
  cast+rowsum pass.  The output add reads bf16 x + f32 psum corr -> f32 out
  tile -> store.  Output = bf16(x) + corr: ~1e-3 rel err, far under the 2e-2
  gate, and it halves SBUF so both items' bf16 tiles coexist (item1's casts
  never wait on item0's projections - the v1 stall).
- Stores live on nc.sync (not nc.scalar): an HWDGE dma_start blocks its
  issuing sequencer until the source tile is ready, which on nc.scalar would
  stall ACT compute behind a waiting store.
- SE gate: php accumulates per-chunk on PE as each cast's rowsum lands
  (during the load phase); relu/gate-matmuls/sigmoid after the last chunk.
  b2 is folded into the gate matmul as an extra stationary row against a
  constant-1 row in hid1, so one sigmoid over [128,4] suffices.
- proj: 36 bf16 matmuls [128x5x512] into 2 psum banks, psum->prow copies on
  DVE (bias add fused).  g.phi dot: prow rows 3,4 are reshaped [1,1536] ->
  [128,12] by SWDGE SBUF-SBUF DMAs in groups as produced (DVE lanes cannot
  cross partitions, so the dot needs the operands re-laid); then ONE fused
  tensor_tensor_reduce (mult+sum) -> r1, cross-partition sum via ones-matmul,
  ab3 row 0 = art * s_raw.
- corr = [As; Bc_hi; Bc_lo].T @ [theta; 1; 1] on PE in 1024-wide psum tiles
  (2 banks x 2 bufs) - fewer instructions than 512-wide; DVE adds
  out = bf16(x) + psum per 1024 block; store per chunk.
- PE emission order matches runtime readiness: php0, gate0, proj0, sb0,
  corr0, php1, gate1, proj1, sb1, corr1.  (In-order engines: emission order
  that diverges from readiness order = stalls.)
"""

import numpy as np

B, C, H, W = 16, 512, 96, 48
N = H * W            # 4608
P = 128
KC = C // P          # 4 channel chunks
NB = 512             # proj free-dim block = one fp32 PSUM bank
NJ = N // NB         # 9
CB = 1024            # corr free-dim block (2 psum banks)
NCORES = 8
BPC = B // NCORES    # 2 batch items per core
SE_C = C // 16       # 32
MR = N // P          # 36: elems per partition in the reshaped g/phi rows
BN_EPS = 1e-5

_CACHE = {}
LAST_RESULTS = None


def _build_bass():
    import concourse.mybir as mybir
    from concourse.bacc import Bacc
    from concourse.tile import TileContext

    f32 = mybir.dt.float32
    bf16 = mybir.dt.bfloat16
    AF = mybir.ActivationFunctionType
    ALU = mybir.AluOpType

    nc = Bacc()
    xs = nc.dram_tensor("xs", [BPC, C, N], f32, kind="ExternalInput")
    w1 = nc.dram_tensor("w1", [P, KC, SE_C], f32, kind="ExternalInput")
    w2e = nc.dram_tensor("w2e", [SE_C + 1, C], f32, kind="ExternalInput")
    b1 = nc.dram_tensor("b1", [SE_C, 1], f32, kind="ExternalInput")
    w5 = nc.dram_tensor("w5", [P, KC, 5], f32, kind="ExternalInput")
    pb = nc.dram_tensor("pb", [5, 1], f32, kind="ExternalInput")
    ar = nc.dram_tensor("ar", [1, C], f32, kind="ExternalInput")    # W_w*inv/N
    bchl = nc.dram_tensor("bchl", [2, C], bf16, kind="ExternalInput")  # Bc hi/lo
    out_d = nc.dram_tensor("out", [BPC, C, N], f32, kind="ExternalOutput")

    with TileContext(nc) as tc:
        with (
            tc.tile_pool(name="wpool", bufs=1) as wpool,
            tc.tile_pool(name="ldpool", bufs=3) as ldpool,
            tc.tile_pool(name="xbpool", bufs=BPC * KC) as xbpool,
            tc.tile_pool(name="opool", bufs=2) as opool,
            tc.tile_pool(name="ppool", bufs=2) as ppool,
            tc.tile_pool(name="spool", bufs=2) as spool,
            tc.tile_pool(name="ps_se", bufs=2, space="PSUM") as ps_se,
            tc.tile_pool(name="ps_pj", bufs=2, space="PSUM") as ps_pj,
            tc.tile_pool(name="ps_cr", bufs=2, space="PSUM") as ps_cr,
        ):
            w1t = wpool.tile([P, KC, SE_C], f32, tag="w1t")
            w2t = wpool.tile([SE_C + 1, C], f32, tag="w2t")
            b1t = wpool.tile([SE_C, 1], f32, tag="b1t")
            w5t = wpool.tile([P, KC, 5], f32, tag="w5t")
            pbt = wpool.tile([5, 1], f32, tag="pbt")
            art = wpool.tile([1, C], f32, tag="art")
            ab3 = wpool.tile([3, C], bf16, tag="ab3")     # rows: As, Bc_hi, Bc_lo
            hid1 = wpool.tile([SE_C + 1, 1], f32, tag="hid1")  # [hid; 1.0]
            on128 = wpool.tile([P, P], f32, tag="on128")  # all-ones (part. sum)

            nc.vector.memset(hid1[SE_C:SE_C + 1, :], 1.0)
            nc.vector.memset(on128[:], 1.0)
            for t, d in ((w1t, w1), (w2t, w2e), (b1t, b1),
                         (w5t, w5), (pbt, pb), (art, ar)):
                nc.gpsimd.dma_start(out=t[:], in_=d[:])
            nc.gpsimd.dma_start(out=ab3[1:3, :], in_=bchl[:])

            # preload the ACT sigmoid table while idle (else the first SE
            # sigmoid pays a ~1.3us ACT_TABLE_LOAD on the critical path)
            dmy = spool.tile([1, 1], f32, tag="dmy", bufs=1)
            nc.vector.memset(dmy[:], 0.0)
            nc.scalar.activation(out=dmy[:], in_=dmy[:], func=AF.Sigmoid)

            # ---- all 8 chunk loads, sequential on the sync ring (chunk 0
            #      lands at full rate; stores queue behind on the same ring)
            xts = []
            for i in range(BPC * KC):
                b, k = divmod(i, KC)
                xt = ldpool.tile([P, N], f32, tag="xt")
                nc.sync.dma_start(out=xt[:], in_=xs[b, k * P:(k + 1) * P, :])
                xts.append(xt)

            xbs = [[None] * KC for _ in range(BPC)]
            xps = [None] * BPC
            prows = [None] * BPC

            def casts(b, with_php):
                # fused bf16 cast + spatial sums on ACT (one pass); php
                # accumulates on PE per chunk (item0: during load phase)
                xp = spool.tile([P, KC], f32, tag="xp")
                xps[b] = xp
                php = (ps_se.tile([SE_C, 1], f32, tag="ps_se", name="php")
                       if with_php else None)
                for k in range(KC):
                    xb = xbpool.tile([P, N], bf16, tag="xb")
                    nc.scalar.activation(out=xb[:], in_=xts[b * KC + k][:],
                                         func=AF.Identity,
                                         accum_out=xp[:, k:k + 1])
                    xbs[b][k] = xb
                    if with_php:
                        nc.tensor.matmul(php[:], w1t[:, k, :], xp[:, k:k + 1],
                                         start=(k == 0), stop=(k == KC - 1))
                return php

            def se_php(b):
                php = ps_se.tile([SE_C, 1], f32, tag="ps_se")
                for k in range(KC):
                    nc.tensor.matmul(php[:], w1t[:, k, :], xps[b][:, k:k + 1],
                                     start=(k == 0), stop=(k == KC - 1))
                return php

            def se_gate(b, php):
                nc.scalar.activation(out=hid1[0:SE_C, :], in_=php[:],
                                     func=AF.Relu, bias=b1t[:], scale=1.0)
                gate = spool.tile([P, KC], f32, tag="gate")
                for k in range(KC):
                    gp = ps_se.tile([P, 1], f32, tag="ps_se", name="gp")
                    nc.tensor.matmul(gp[:], w2t[:, k * P:(k + 1) * P],
                                     hid1[:], start=True, stop=True)
                    nc.scalar.activation(out=gate[:, k:k + 1], in_=gp[:],
                                         func=AF.Sigmoid)
                w5e = spool.tile([P, KC, 5], bf16, tag="w5e")
                for k in range(KC):
                    nc.vector.tensor_scalar_mul(out=w5e[:, k, :],
                                                in0=w5t[:, k, :],
                                                scalar1=gate[:, k:k + 1])
                return w5e

            def proj_and_dot(b, w5e):
                # prow = w5e.T @ x (bf16 PE); psum->prow copies on DVE with
                # the bias fused; g/phi rows stream into [128, .] layout as
                # they are produced (SWDGE; same n-permutation for both rows)
                prow = ppool.tile([5, N], bf16, tag="prow")
                prows[b] = prow
                g_rs = spool.tile([P, MR], bf16, tag="g_rs")
                p_rs = spool.tile([P, MR], bf16, tag="p_rs")
                for j in range(NJ):
                    pp = ps_pj.tile([5, NB], f32, tag="pp")
                    for k in range(KC):
                        nc.tensor.matmul(pp[:], w5e[:, k, :],
                                         xbs[b][k][:, j * NB:(j + 1) * NB],
                                         start=(k == 0), stop=(k == KC - 1))
                    nc.vector.tensor_scalar_add(
                        out=prow[:, j * NB:(j + 1) * NB],
                        in0=pp[:], scalar1=pbt[:])
                    if j in (3, 7, 8):
                        lo = {3: 0, 7: 4, 8: 8}[j]
                        nsl = slice(lo * NB, (j + 1) * NB)
                        msl = slice(lo * (NB // P), (j + 1) * (NB // P))
                        nc.gpsimd.dma_start(out=g_rs[:, msl],
                                            in_=prow[3:4, nsl])
                        nc.gpsimd.dma_start(out=p_rs[:, msl],
                                            in_=prow[4:5, nsl])
                # dot: prod = g*phi, r1 = rowsum(prod)
                prod = spool.tile([P, MR], f32, tag="prod")
                r1 = spool.tile([P, 1], f32, tag="r1")
                nc.vector.tensor_mul(out=prod[:], in0=g_rs[:], in1=p_rs[:])
                nc.vector.reduce_sum(out=r1[:], in_=prod[:],
                                     axis=mybir.AxisListType.X)
                sb = ps_se.tile([P, 1], f32, tag="ps_se")
                nc.tensor.matmul(sb[:], on128[:], r1[:], start=True, stop=True)
                # As row = (W_w*inv/N) * s_raw, into ab3 row 0 (bf16)
                nc.vector.tensor_scalar_mul(out=ab3[0:1, :], in0=art[:],
                                            scalar1=sb[0:1, 0:1])

            def corr_and_store(b):
                # corr = As x theta + Bc x ones via PE (3-row bf16, 1024-wide
                # psum); out = bf16(x) + corr on DVE; store per chunk
                prow = prows[b]
                for k in range(KC):
                    ot = opool.tile([P, N], f32, tag="ot")
                    for n0 in range(0, N, CB):
                        gw = min(CB, N - n0)
                        cp = ps_cr.tile([P, CB], f32, tag="cp")
                        # matmul output must stay within one psum bank:
                        # two 512-wide matmuls, one 1024-wide DVE add
                        for m0 in range(0, gw, NB):
                            nc.tensor.matmul(cp[:, m0:m0 + NB],
                                             ab3[:, k * P:(k + 1) * P],
                                             prow[0:3, n0 + m0:n0 + m0 + NB],
                                             start=True, stop=True)
                        nc.vector.tensor_add(out=ot[:, n0:n0 + gw],
                                             in0=xbs[b][k][:, n0:n0 + gw],
                                             in1=cp[:, 0:gw])
                    nc.sync.dma_start(out=out_d[b, k * P:(k + 1) * P, :],
                                      in_=ot[:])

            # ---- item 0: head overlaps the load phase ----
            php0 = casts(0, with_php=True)
            w5e0 = se_gate(0, php0)
            proj_and_dot(0, w5e0)
            casts(1, with_php=False)     # ACT: item1 casts right after sig0
            corr_and_store(0)
            # ---- item 1: tail (SE matmuls emitted after corr0 on PE) ----
            php1 = se_php(1)
            w5e1 = se_gate(1, php1)
            proj_and_dot(1, w5e1)
            corr_and_store(1)

    nc.finalize()
    return nc


def kernel(**inputs):
    global LAST_RESULTS
    from concourse.bass_utils import run_bass_kernel_spmd
    import ml_dtypes

    a = {k: np.asarray(v, dtype=np.float32) for k, v in inputs.items()}
    x = np.ascontiguousarray(a["x"]).reshape(B, C, N)

    inv = a["bn_gamma"] / np.sqrt(a["bn_var"] + BN_EPS)
    A = (a["W_w"] * inv / N).astype(np.float32)
    Bc = ((a["W_b"] - a["bn_mean"]) * inv + a["bn_beta"]).astype(np.float32)
    Bc_hi = Bc.astype(ml_dtypes.bfloat16)
    Bc_lo = (Bc - Bc_hi.astype(np.float32)).astype(ml_dtypes.bfloat16)

    w1h = np.ascontiguousarray(
        (a["se_w1"] / N).T.reshape(KC, P, SE_C).transpose(1, 0, 2)).astype(np.float32)
    w2h = np.ascontiguousarray(
        np.vstack([a["se_w2"].T, a["se_b2"][None, :]])).astype(np.float32)
    b1h = np.ascontiguousarray(a["se_b1"].reshape(SE_C, 1))
    zc = np.zeros(C, np.float32)
    w5h = np.ascontiguousarray(
        np.stack([a["theta_w"], zc, zc, a["g_w"], a["phi_w"]],
                 axis=1).reshape(KC, P, 5).transpose(1, 0, 2)).astype(np.float32)
    pbh = np.array([[a["theta_b"]], [1.0], [1.0], [a["g_b"]], [a["phi_b"]]],
                   dtype=np.float32)
    arh = np.ascontiguousarray(A.reshape(1, C))
    bchlh = np.ascontiguousarray(np.stack([Bc_hi, Bc_lo], axis=0))

    if "nc" not in _CACHE:
        _CACHE["nc"] = _build_bass()
    nc = _CACHE["nc"]

    in_maps = []
    for c in range(NCORES):
        in_maps.append({
            "xs": np.ascontiguousarray(x[c * BPC:(c + 1) * BPC]),
            "w1": w1h, "w2e": w2h, "b1": b1h,
            "w5": w5h, "pb": pbh, "ar": arh, "bchl": bchlh,
        })

    res = run_bass_kernel_spmd(nc, in_maps, core_ids=list(range(NCORES)))
    LAST_RESULTS = res

    out = np.concatenate([res.results[c]["out"] for c in range(NCORES)], axis=0)
    return np.ascontiguousarray(out.reshape(B, C, H, W))
